# revision 42
# baseline (speedup 1.0000x reference)
"""Trainium2 Bass kernel: LinearSelfAttentionTemporal (N,C,T,V)=(64,128,64,25).

Data-parallel over batch N across 8 NeuronCores (8 samples each).
Per sample the pipeline runs in the natural (C=128 partitions, L=T*V=1600
free) layout:
  - c_attn / c_proj 1x1 convs as PE matmuls contracting over C
  - cumulative sums via DVE tensor_tensor_scan along the free dim
  - softmax WITHOUT max-subtraction: logits = temp*sum_hd(wn) are in
    [0, 16*temp] (wn = wsq/denom <= 1 since denom is an inclusive cumsum),
    so exp() is safe in fp32; denom_bias adds a per-(n,h) constant to the
    logits and cancels exactly in softmax, so it is dropped.
  - per-head (8 -> 128 partition) broadcasts via 0-stride DMA replication
  - samples processed in PAIRS: elementwise ops run on (C, 2L) fused tiles
    to halve instruction count (and Pool-engine semaphore overhead); scans
    and PSUM epilogues stay per-sample on slices of the pair tiles.
Algebra: with Pi = softmax(tmp), A = cumsum(Pi)+1e-8,
  dots = cumsum(wsq*Pi)/A  =>  attn = 1/(1+dots) = A/D
  where D = 1e-8 + cumsum((wsq+1)*Pi)  (scan with data1=Pi fused add)
  y = -(w*Pi)*attn = -(w * (Pi*A)_bcast) / D   (minus folded into -Wp^T)
The reference denom clamp max(cumsum(wsq), 1e-12) is reproduced exactly by
the scan recurrence state=(wsq+state) max 1e-12 (error <= 1e-12 absolute).
"""
import os
import sys

import numpy as np

for _p in ("/opt/trn_rl_repo",):
    if _p not in sys.path and os.path.isdir(_p):
        sys.path.insert(0, _p)

import ml_dtypes
import concourse.bacc as bacc
import concourse.tile as tile
from concourse import mybir
from concourse.bass_utils import run_bass_kernel_spmd

F32 = mybir.dt.float32
BF16 = mybir.dt.bfloat16
FP16 = mybir.dt.float16
AOP = mybir.AluOpType
AFT = mybir.ActivationFunctionType

N, C, T, V = 64, 128, 64, 25
H, HD, L = 8, 16, T * V
L2 = 2 * L
NCORES = 8
NLOC = N // NCORES
G, GS = 2, NLOC // 2  # two groups of 4 samples; 2 pairs per group
# halves of L for the 2-bank psum tiles: (offset, width, sub-chunks)
HALVES = [(0, 1024, [(0, 512), (512, 512)]), (1024, 576, [(0, 512), (512, 64)])]
CHUNKS = [(0, 512), (512, 512), (1024, 512), (1536, 64)]

DEFAULT_CFG = dict(
    # GpSimd (Pool) tensor ops measured 3.3-6.6us vs DVE 1.0us; DVE carries
    # the critical elementwise chain. The scalar (Activation) queue paces the
    # softmax, so wsq moves to the otherwise-idle GpSimd as w*w.
    wn_eng="d",    # wn = wsq*rden
    v2_eng="d",    # v2 = w*u_b
    p2m_eng="d",   # p2m = wsq*PiB
    y_eng="d",     # y = v2*rD
    wsq_eng="g",   # wsq: "s" scalar Square-act from psum | "g" gpsimd w*w
)


def _act_recip(nc, out, in_):
    """Scalar-engine Reciprocal activation (HW-verified ~1.2e-5 rel err for
    normal-range inputs; inputs here are clamped >= 1e-12)."""
    ins = [nc.scalar.lower_ap(in_)]
    for arg in (0.0, 1.0, 0.0):  # bias, scale, alpha immediates
        ins.append(mybir.ImmediateValue(dtype=mybir.dt.float32, value=arg))
    return nc.scalar.add_instruction(
        mybir.InstActivation(
            name=nc.get_next_instruction_name(),
            func=mybir.ActivationFunctionType.Reciprocal,
            ins=ins,
            outs=[nc.scalar.lower_ap(out)],
        )
    )


def build_nc(cfg=None):
    """Build and compile the per-core Bass program. Returns nc."""
    cfg = {**DEFAULT_CFG, **(cfg or {})}
    from contextlib import ExitStack

    nc = bacc.Bacc("TRN2", target_bir_lowering=False, debug=False)

    x_d = nc.dram_tensor("x16", (NLOC, C, L), FP16, kind="ExternalInput").ap()
    wat_d = nc.dram_tensor("wat16", (C, C), FP16, kind="ExternalInput").ap()
    wptn_d = nc.dram_tensor("wptn_bf", (C, C), BF16, kind="ExternalInput").ap()
    iden_d = nc.dram_tensor("iden16", (C, C), FP16, kind="ExternalInput").ap()
    ba_d = nc.dram_tensor("ba", (C, 1), F32, kind="ExternalInput").ap()
    bp_d = nc.dram_tensor("bp", (C, 1), F32, kind="ExternalInput").ap()
    m64_d = nc.dram_tensor("m64bf", (C, NLOC * 32), BF16, kind="ExternalInput").ap()
    sc_d = nc.dram_tensor("sc64", (64, 1), F32, kind="ExternalInput").ap()
    out_d = nc.dram_tensor("out16", (NLOC, C, L), FP16, kind="ExternalOutput").ap()

    def eng_of(key):
        return nc.gpsimd if cfg[key] == "g" else nc.vector

    with tile.TileContext(nc) as tc, ExitStack() as ctx:
        cons = ctx.enter_context(tc.tile_pool(name="consts", bufs=1))
        xpool = ctx.enter_context(tc.tile_pool(name="xp", bufs=1))
        wpool = ctx.enter_context(tc.tile_pool(name="wp", bufs=2))
        sqpool = ctx.enter_context(tc.tile_pool(name="sqp", bufs=2))
        work = ctx.enter_context(tc.tile_pool(name="wk", bufs=2))
        soft = ctx.enter_context(tc.tile_pool(name="sf", bufs=2))
        opool = ctx.enter_context(tc.tile_pool(name="op", bufs=2))
        pspool = ctx.enter_context(tc.tile_pool(name="ps", bufs=1, space="PSUM"))

        # first-needed consts on the sync ring, the rest via gpsimd so the
        # first c_attn isn't stuck behind unrelated const loads
        wat_s = cons.tile([C, C], FP16)
        nc.sync.dma_start(wat_s[:], wat_d[:])
        ba_s = cons.tile([C, 1], F32)
        nc.sync.dma_start(ba_s[:], ba_d[:])
        wptn_s = cons.tile([C, C], BF16)
        nc.gpsimd.dma_start(wptn_s[:], wptn_d[:])
        iden_s = cons.tile([C, C], FP16)
        nc.gpsimd.dma_start(iden_s[:], iden_d[:])
        bp_s = cons.tile([C, 1], F32)
        nc.gpsimd.dma_start(bp_s[:], bp_d[:])
        m64_s = cons.tile([C, NLOC * 32], BF16)
        nc.gpsimd.dma_start(m64_s[:], m64_d[:])
        sc_s = cons.tile([64, 1], F32)
        nc.gpsimd.dma_start(sc_s[:], sc_d[:])
        eps_c = cons.tile([C, 1], BF16)
        nc.gpsimd.memset(eps_c[:], 1e-12)
        epsC = eps_c[:].broadcast_to((C, L))
        z32 = cons.tile([32, 1], BF16)
        nc.gpsimd.memset(z32[:], 0.0)
        z32L = z32[:].broadcast_to((32, L))

        # all of x stays resident: (C, 8L) fp16 = 25.6KB/partition. Loads go
        # on the scalar HW-DGE ring, which is otherwise idle this early, so
        # the sync/gpsimd rings stay clear for the head-broadcasts.
        x_all = xpool.tile([C, NLOC * L], FP16)
        nc.scalar.dma_start(x_all[:, 0:L], x_d[0])
        for n in range(1, NLOC):
            nc.sync.dma_start(x_all[:, n * L : (n + 1) * L], x_d[n])

        w_l = [None] * NLOC
        wsq_l = [None] * NLOC
        wn_l = [None] * NLOC
        # per-group chunk-A tmp psums (both groups alive: 2 tiles x 2 banks)
        ptmpA = [
            pspool.tile([32, 1024], F32, tag="ptmp", bufs=2, name=f"ptmpA{g}")
            for g in range(G)
        ]

        e_l = [None] * G
        sA_l = [None] * G
        sB_l = [None] * G

        def a_one(n):
            """c_attn + epilogues + denom pipeline + chunk-A tmp-matmuls."""
            j = n % GS
            g = n // GS
            w_t = wpool.tile([C, L], BF16, tag="w", bufs=NLOC)
            wsq_t = sqpool.tile([C, L], BF16, tag="wsq", bufs=NLOC)
            w_l[n] = w_t
            wsq_l[n] = wsq_t
            o_n = n * L
            for (ho, hw, subs) in HALVES:
                pw = pspool.tile([C, 1024], F32, tag="pp", bufs=2)
                for (so, sw) in subs:
                    nc.tensor.matmul(
                        pw[:, so : so + sw],
                        wat_s[:],
                        x_all[:, o_n + ho + so : o_n + ho + so + sw],
                        start=True,
                        stop=True,
                    )
                if cfg["wsq_eng"] == "s":
                    nc.scalar.activation(
                        wsq_t[:, ho : ho + hw], pw[:, 0:hw], AFT.Square, bias=ba_s[:]
                    )
                nc.scalar.activation(
                    w_t[:, ho : ho + hw], pw[:, 0:hw], AFT.Identity, bias=ba_s[:]
                )
            if cfg["wsq_eng"] == "g":
                nc.gpsimd.tensor_tensor(wsq_t[:], w_t[:], w_t[:], AOP.mult)
            den_t = work.tile([C, L], BF16, tag="den", bufs=2)
            nc.vector.tensor_tensor_scan(
                den_t[:], wsq_t[:], epsC, 0.0, AOP.add, AOP.max
            )
            rden_t = work.tile([C, L], BF16, tag="rden", bufs=2)
            _act_recip(nc, rden_t[:], den_t[:])
            wn_t = work.tile([C, L], BF16, tag="wn", bufs=6)
            eng_of("wn_eng").tensor_tensor(wn_t[:], wsq_t[:], rden_t[:], AOP.mult)
            wn_l[n] = wn_t
            # tmp chunk A (L-cols 0:1024) accumulates into this group's ptmpA
            pta = ptmpA[g]
            for (o, cw) in [(0, 512), (512, 512)]:
                nc.tensor.matmul(
                    pta[0:32, o : o + cw],
                    m64_s[:, n * 32 : (n + 1) * 32],
                    wn_t[:, o : o + cw],
                    start=(j == 0),
                    stop=(j == GS - 1),
                )

        def exp_a(g):
            """exp over tmp chunk A straight from psum (no max subtraction)."""
            r0, r1 = g * 32, (g + 1) * 32
            e_g = soft.tile([32, L], BF16, tag="e", bufs=2)
            sA = soft.tile([32, 1], F32, tag="sA", bufs=2)
            e_l[g], sA_l[g] = e_g, sA
            nc.scalar.activation(
                e_g[:, 0:1024],
                ptmpA[g][0:32, 0:1024],
                AFT.Exp,
                scale=sc_s[r0:r1, :],
                accum_out=sA[:],
            )

        def tmp_b_wave(g):
            """tmp chunk B (L-cols 1024:1600) matmuls + exp."""
            r0, r1 = g * 32, (g + 1) * 32
            ptb = pspool.tile([32, 1024], F32, tag="ptmp", bufs=2)
            for j in range(GS):
                n = g * GS + j
                for (o, cw, po) in [(1024, 512, 0), (1536, 64, 512)]:
                    nc.tensor.matmul(
                        ptb[0:32, po : po + cw],
                        m64_s[:, n * 32 : (n + 1) * 32],
                        wn_l[n][:, o : o + cw],
                        start=(j == 0),
                        stop=(j == GS - 1),
                    )
            sB = soft.tile([32, 1], F32, tag="sB", bufs=2)
            sB_l[g] = sB
            nc.scalar.activation(
                e_l[g][:, 1024:L],
                ptb[0:32, 0:576],
                AFT.Exp,
                scale=sc_s[r0:r1, :],
                accum_out=sB[:],
            )

        def b_small(g):
            """Softmax epilogue in the unnormalized-e domain.

            With the 1e-8 regularizers dropped (they perturb y only where
            cumPi <~ 1e-6, i.e. |y| <~ |w|*1e-6 -- far below tolerance):
              y = w*Pi*cumPi/D = w*e*t_b/D0hat,
              t = e*cumE*rs,  D0hat = cumsum((1+wsq)*e)  (c-side scan).
            So the c-side needs only e (early) and t (late) -- broadcast
            separately so the c-chain starts as soon as e is ready."""
            e_g = e_l[g]
            s_g = soft.tile([32, 1], F32, tag="s", bufs=2)
            nc.vector.tensor_tensor(s_g[:], sA_l[g][:], sB_l[g][:], AOP.add)
            rs = soft.tile([32, 1], F32, tag="rs", bufs=2)
            nc.vector.reciprocal(rs[:], s_g[:])
            t_g = soft.tile([32, L], BF16, tag="t", bufs=2)
            nc.vector.tensor_tensor_scan(t_g[:], e_g[:], z32L, 0.0, AOP.add, AOP.add)
            nc.vector.tensor_scalar_mul(t_g[:], t_g[:], rs[:])
            nc.vector.tensor_tensor(t_g[:], t_g[:], e_g[:], AOP.mult)
            return t_g

        eb_l = [None] * NLOC
        tb_l = [None] * NLOC
        y_l = [None] * NLOC

        def c_bcast_e(n):
            """Early head-broadcast of e for sample n (feeds p2m + D-scan).
            Dedicated sync DGE ring so e never queues behind t/out traffic."""
            r = 8 * (n % GS)
            e_g = e_l[n // GS]
            eb = work.tile([C, L], BF16, tag="eb", bufs=6)
            eng = (nc.sync, nc.scalar, nc.gpsimd)[n % 3]
            eng.dma_start(
                eb[:], e_g[r : r + 8, :].unsqueeze(1).broadcast_to((8, HD, L))
            )
            eb_l[n] = eb

        def c_bcast_t(n, t_g):
            """Late head-broadcast of t = e*cumE/s (feeds v2 only)."""
            r = 8 * (n % GS)
            tb = work.tile([C, L], BF16, tag="tb", bufs=6)
            eng = (nc.gpsimd, nc.sync, nc.scalar)[n % 3]
            eng.dma_start(
                tb[:], t_g[r : r + 8, :].unsqueeze(1).broadcast_to((8, HD, L))
            )
            tb_l[n] = tb

        rD_l = [None] * NLOC

        def c_pd(n):
            """p2m + D-scan + reciprocal for sample n (needs only eb)."""
            Eb = eb_l[n][:]
            p2m = work.tile([C, L], BF16, tag="p2m", bufs=2)
            eng_of("p2m_eng").tensor_tensor(p2m[:], wsq_l[n][:], Eb, AOP.mult)
            D_t = work.tile([C, L], BF16, tag="D", bufs=2)
            nc.vector.tensor_tensor_scan(
                D_t[:], p2m[:], Eb, 0.0, AOP.add, AOP.add
            )
            rD = work.tile([C, L], BF16, tag="rD", bufs=3)
            _act_recip(nc, rD[:], D_t[:])
            rD_l[n] = rD

        def c_vy(n):
            """v2 + y for sample n (needs tb, which arrives later)."""
            v2 = work.tile([C, L], BF16, tag="v2", bufs=2)
            eng_of("v2_eng").tensor_tensor(v2[:], w_l[n][:], tb_l[n][:], AOP.mult)
            y_t = work.tile([C, L], BF16, tag="y", bufs=3)
            eng_of("y_eng").tensor_tensor(y_t[:], v2[:], rD_l[n][:], AOP.mult)
            y_l[n] = y_t

        def c_out(n):
            """Projection + residual + relu + store for sample n."""
            y_t = y_l[n]
            out_sb = opool.tile([C, L], FP16, tag="outsb", bufs=2)
            o_n = n * L
            for (ho, hw, subs) in HALVES:
                pj = pspool.tile([C, 1024], F32, tag="pp", bufs=2)
                for (so, sw) in subs:
                    nc.tensor.matmul(
                        pj[:, so : so + sw],
                        wptn_s[:],
                        y_t[:, ho + so : ho + so + sw],
                        start=True,
                        stop=False,
                    )
                for (so, sw) in subs:
                    nc.tensor.matmul(
                        pj[:, so : so + sw],
                        iden_s[:],
                        x_all[:, o_n + ho + so : o_n + ho + so + sw],
                        start=False,
                        stop=True,
                    )
                nc.scalar.activation(
                    out_sb[:, ho : ho + hw], pj[:, 0:hw], AFT.Relu, bias=bp_s[:]
                )
            eng = nc.sync if n % 2 == 0 else nc.gpsimd
            eng.dma_start(out_d[n], out_sb[:])

        # Software-pipelined emission in data-readiness order: all A-phases
        # lead; each group's softmax/broadcast chain threads between later
        # A-pieces so broadcast latency always overlaps independent work,
        # and c-chains trail where their inputs become ready.
        for n in range(5):
            a_one(n)
        exp_a(0)
        a_one(5)
        tmp_b_wave(0)
        a_one(6)
        for n in range(4):
            c_bcast_e(n)
        t0 = b_small(0)
        for n in range(4):
            c_bcast_t(n, t0)
        a_one(7)
        exp_a(1)
        tmp_b_wave(1)
        for n in range(4, 8):
            c_bcast_e(n)
        t1 = b_small(1)
        for n in range(4, 8):
            c_bcast_t(n, t1)
        c_pd(0)
        c_pd(1)
        c_vy(0)
        c_pd(2)
        c_vy(1)
        c_out(0)
        c_pd(3)
        c_vy(2)
        c_out(1)
        c_pd(4)
        c_vy(3)
        c_out(2)
        c_pd(5)
        c_vy(4)
        c_out(3)
        c_pd(6)
        c_vy(5)
        c_out(4)
        c_pd(7)
        c_vy(6)
        c_out(5)
        c_vy(7)
        c_out(6)
        c_out(7)

    nc.compile()
    return nc


def make_core_inputs(inputs, cfg=None):
    """Host-side prep: returns (shared_map, per_core_x_list)."""
    x = np.asarray(inputs["x"], np.float32)  # (N,C,T,V)
    Wa = np.asarray(inputs["Wa"], np.float32)
    ba = np.asarray(inputs["ba"], np.float32)
    Wp = np.asarray(inputs["Wp"], np.float32)
    bp = np.asarray(inputs["bp"], np.float32)
    temp = np.asarray(inputs["temp"], np.float32).reshape(H)
    # denom_bias adds a per-(n,h) constant to the softmax logits -> cancels.

    assert np.all(temp > 0), "kernel assumes temp > 0"
    assert temp.max() * 16.0 < 80.0, "kernel assumes exp(16*temp) fits fp32"

    xr = np.ascontiguousarray(x.reshape(N, C, L).astype(np.float16))
    wat16 = np.ascontiguousarray(Wa.T).astype(np.float16)
    wptn_bf = np.ascontiguousarray((-Wp.T)).astype(ml_dtypes.bfloat16)
    iden16 = np.eye(C, dtype=np.float16)
    m64 = np.zeros((C, NLOC * 32), np.float32)
    cc = np.arange(C)
    for n in range(NLOC):
        m64[cc, n * 32 + 8 * (n % GS) + cc // HD] = 1.0
    m64bf = m64.astype(ml_dtypes.bfloat16)
    pp = np.arange(64)
    sc64 = temp[pp % 8].reshape(64, 1).astype(np.float32)

    shared = dict(
        wat16=wat16,
        wptn_bf=wptn_bf,
        iden16=iden16,
        ba=ba.reshape(C, 1),
        bp=bp.reshape(C, 1),
        m64bf=m64bf,
        sc64=sc64,
    )
    xs = [np.ascontiguousarray(xr[i * NLOC : (i + 1) * NLOC]) for i in range(NCORES)]
    return shared, xs


_NC_CACHE = {}


def kernel(**inputs):
    cfg_key = "default"
    if cfg_key not in _NC_CACHE:
        _NC_CACHE[cfg_key] = build_nc()
    nc = _NC_CACHE[cfg_key]
    shared, xs = make_core_inputs(inputs)
    in_maps = [dict(shared, x16=xs[i]) for i in range(NCORES)]
    res = run_bass_kernel_spmd(nc, in_maps, core_ids=list(range(NCORES)))
    out = np.concatenate([res.results[i]["out16"] for i in range(NCORES)], axis=0)
    return out.reshape(N, C, T, V).astype(np.float32)


if __name__ == "__main__":
    rng = np.random.default_rng(0)
    demo = dict(
        x=rng.standard_normal((N, C, T, V)).astype(np.float32),
        Wa=rng.standard_normal((C, C)).astype(np.float32) / np.sqrt(C),
        ba=rng.standard_normal((C,)).astype(np.float32) * 0.01,
        Wp=rng.standard_normal((C, C)).astype(np.float32) / np.sqrt(C),
        bp=rng.standard_normal((C,)).astype(np.float32) * 0.01,
        temp=np.ones((H, 1), np.float32),
        denom_bias=np.zeros((H, 1, 1), np.float32),
    )
    o = kernel(**demo)
    print("out", o.shape, o.dtype, float(np.abs(o).max()))


# revision 43
# speedup vs baseline: 1.0065x; 1.0065x over previous
"""Trainium2 Bass kernel: LinearSelfAttentionTemporal (N,C,T,V)=(64,128,64,25).

Data-parallel over batch N across 8 NeuronCores (8 samples each).
Per sample the pipeline runs in the natural (C=128 partitions, L=T*V=1600
free) layout:
  - c_attn / c_proj 1x1 convs as PE matmuls contracting over C
  - cumulative sums via DVE tensor_tensor_scan along the free dim
  - softmax WITHOUT max-subtraction: logits = temp*sum_hd(wn) are in
    [0, 16*temp] (wn = wsq/denom <= 1 since denom is an inclusive cumsum),
    so exp() is safe in fp32; denom_bias adds a per-(n,h) constant to the
    logits and cancels exactly in softmax, so it is dropped.
  - per-head (8 -> 128 partition) broadcasts via 0-stride DMA replication
  - samples processed in PAIRS: elementwise ops run on (C, 2L) fused tiles
    to halve instruction count (and Pool-engine semaphore overhead); scans
    and PSUM epilogues stay per-sample on slices of the pair tiles.
Algebra: with Pi = softmax(tmp), A = cumsum(Pi)+1e-8,
  dots = cumsum(wsq*Pi)/A  =>  attn = 1/(1+dots) = A/D
  where D = 1e-8 + cumsum((wsq+1)*Pi)  (scan with data1=Pi fused add)
  y = -(w*Pi)*attn = -(w * (Pi*A)_bcast) / D   (minus folded into -Wp^T)
The reference denom clamp max(cumsum(wsq), 1e-12) is reproduced exactly by
the scan recurrence state=(wsq+state) max 1e-12 (error <= 1e-12 absolute).
"""
import os
import sys

import numpy as np

for _p in ("/opt/trn_rl_repo",):
    if _p not in sys.path and os.path.isdir(_p):
        sys.path.insert(0, _p)

import ml_dtypes
import concourse.bacc as bacc
import concourse.tile as tile
from concourse import mybir
from concourse.bass_utils import run_bass_kernel_spmd

F32 = mybir.dt.float32
BF16 = mybir.dt.bfloat16
FP16 = mybir.dt.float16
AOP = mybir.AluOpType
AFT = mybir.ActivationFunctionType

N, C, T, V = 64, 128, 64, 25
H, HD, L = 8, 16, T * V
L2 = 2 * L
NCORES = 8
NLOC = N // NCORES
G, GS = 2, NLOC // 2  # two groups of 4 samples; 2 pairs per group
# halves of L for the 2-bank psum tiles: (offset, width, sub-chunks)
HALVES = [(0, 1024, [(0, 512), (512, 512)]), (1024, 576, [(0, 512), (512, 64)])]
CHUNKS = [(0, 512), (512, 512), (1024, 512), (1536, 64)]

DEFAULT_CFG = dict(
    # GpSimd (Pool) tensor ops measured 3.3-6.6us vs DVE 1.0us; DVE carries
    # the critical elementwise chain. The scalar (Activation) queue paces the
    # softmax, so wsq moves to the otherwise-idle GpSimd as w*w.
    wn_eng="d",    # wn = wsq*rden
    v2_eng="d",    # v2 = w*u_b
    p2m_eng="d",   # p2m = wsq*PiB
    y_eng="d",     # y = v2*rD
    wsq_eng="g",   # wsq: "s" scalar Square-act from psum | "g" gpsimd w*w
)


def _act_recip(nc, out, in_):
    """Scalar-engine Reciprocal activation (HW-verified ~1.2e-5 rel err for
    normal-range inputs; inputs here are clamped >= 1e-12)."""
    ins = [nc.scalar.lower_ap(in_)]
    for arg in (0.0, 1.0, 0.0):  # bias, scale, alpha immediates
        ins.append(mybir.ImmediateValue(dtype=mybir.dt.float32, value=arg))
    return nc.scalar.add_instruction(
        mybir.InstActivation(
            name=nc.get_next_instruction_name(),
            func=mybir.ActivationFunctionType.Reciprocal,
            ins=ins,
            outs=[nc.scalar.lower_ap(out)],
        )
    )


def build_nc(cfg=None):
    """Build and compile the per-core Bass program. Returns nc."""
    cfg = {**DEFAULT_CFG, **(cfg or {})}
    from contextlib import ExitStack

    nc = bacc.Bacc("TRN2", target_bir_lowering=False, debug=False)

    x_d = nc.dram_tensor("x16", (NLOC, C, L), FP16, kind="ExternalInput").ap()
    wat_d = nc.dram_tensor("wat16", (C, C), FP16, kind="ExternalInput").ap()
    wptn_d = nc.dram_tensor("wptn_bf", (C, C), BF16, kind="ExternalInput").ap()
    iden_d = nc.dram_tensor("iden16", (C, C), FP16, kind="ExternalInput").ap()
    ba_d = nc.dram_tensor("ba", (C, 1), F32, kind="ExternalInput").ap()
    bp_d = nc.dram_tensor("bp", (C, 1), F32, kind="ExternalInput").ap()
    m64_d = nc.dram_tensor("m64bf", (C, NLOC * 32), BF16, kind="ExternalInput").ap()
    sc_d = nc.dram_tensor("sc64", (64, 1), F32, kind="ExternalInput").ap()
    out_d = nc.dram_tensor("out16", (NLOC, C, L), FP16, kind="ExternalOutput").ap()

    def eng_of(key):
        return nc.gpsimd if cfg[key] == "g" else nc.vector

    with tile.TileContext(nc) as tc, ExitStack() as ctx:
        cons = ctx.enter_context(tc.tile_pool(name="consts", bufs=1))
        xpool = ctx.enter_context(tc.tile_pool(name="xp", bufs=1))
        wpool = ctx.enter_context(tc.tile_pool(name="wp", bufs=2))
        sqpool = ctx.enter_context(tc.tile_pool(name="sqp", bufs=2))
        work = ctx.enter_context(tc.tile_pool(name="wk", bufs=2))
        soft = ctx.enter_context(tc.tile_pool(name="sf", bufs=2))
        opool = ctx.enter_context(tc.tile_pool(name="op", bufs=2))
        pspool = ctx.enter_context(tc.tile_pool(name="ps", bufs=1, space="PSUM"))

        # first-needed consts on the sync ring, the rest via gpsimd so the
        # first c_attn isn't stuck behind unrelated const loads
        wat_s = cons.tile([C, C], FP16)
        nc.sync.dma_start(wat_s[:], wat_d[:])
        ba_s = cons.tile([C, 1], F32)
        nc.sync.dma_start(ba_s[:], ba_d[:])
        wptn_s = cons.tile([C, C], BF16)
        nc.gpsimd.dma_start(wptn_s[:], wptn_d[:])
        iden_s = cons.tile([C, C], FP16)
        nc.gpsimd.dma_start(iden_s[:], iden_d[:])
        bp_s = cons.tile([C, 1], F32)
        nc.gpsimd.dma_start(bp_s[:], bp_d[:])
        m64_s = cons.tile([C, NLOC * 32], BF16)
        nc.gpsimd.dma_start(m64_s[:], m64_d[:])
        sc_s = cons.tile([64, 1], F32)
        nc.gpsimd.dma_start(sc_s[:], sc_d[:])
        eps_c = cons.tile([C, 1], BF16)
        nc.gpsimd.memset(eps_c[:], 1e-12)
        epsC = eps_c[:].broadcast_to((C, L))
        z32 = cons.tile([32, 1], BF16)
        nc.gpsimd.memset(z32[:], 0.0)
        z32L = z32[:].broadcast_to((32, L))

        # all of x stays resident: (C, 8L) fp16 = 25.6KB/partition. Loads go
        # on the scalar HW-DGE ring, which is otherwise idle this early, so
        # the sync/gpsimd rings stay clear for the head-broadcasts.
        x_all = xpool.tile([C, NLOC * L], FP16)
        nc.scalar.dma_start(x_all[:, 0:L], x_d[0])
        for n in range(1, NLOC):
            nc.sync.dma_start(x_all[:, n * L : (n + 1) * L], x_d[n])

        w_l = [None] * NLOC
        wsq_l = [None] * NLOC
        wn_l = [None] * NLOC
        # per-group chunk-A tmp psums (both groups alive: 2 tiles x 2 banks)
        ptmpA = [
            pspool.tile([32, 1024], F32, tag="ptmp", bufs=2, name=f"ptmpA{g}")
            for g in range(G)
        ]

        e_l = [None] * G
        sA_l = [None] * G
        sB_l = [None] * G

        def a_one(n):
            """c_attn + epilogues + denom pipeline + chunk-A tmp-matmuls."""
            j = n % GS
            g = n // GS
            w_t = wpool.tile([C, L], BF16, tag="w", bufs=NLOC)
            wsq_t = sqpool.tile([C, L], BF16, tag="wsq", bufs=NLOC)
            w_l[n] = w_t
            wsq_l[n] = wsq_t
            o_n = n * L
            for (ho, hw, subs) in HALVES:
                pw = pspool.tile([C, 1024], F32, tag="pp", bufs=2)
                for (so, sw) in subs:
                    nc.tensor.matmul(
                        pw[:, so : so + sw],
                        wat_s[:],
                        x_all[:, o_n + ho + so : o_n + ho + so + sw],
                        start=True,
                        stop=True,
                    )
                if cfg["wsq_eng"] == "s":
                    nc.scalar.activation(
                        wsq_t[:, ho : ho + hw], pw[:, 0:hw], AFT.Square, bias=ba_s[:]
                    )
                nc.scalar.activation(
                    w_t[:, ho : ho + hw], pw[:, 0:hw], AFT.Identity, bias=ba_s[:]
                )
            if cfg["wsq_eng"] == "g":
                nc.gpsimd.tensor_tensor(wsq_t[:], w_t[:], w_t[:], AOP.mult)
            elif cfg["wsq_eng"] == "d":
                nc.vector.tensor_tensor(wsq_t[:], w_t[:], w_t[:], AOP.mult)
            den_t = work.tile([C, L], BF16, tag="den", bufs=2)
            nc.vector.tensor_tensor_scan(
                den_t[:], wsq_t[:], epsC, 0.0, AOP.add, AOP.max
            )
            rden_t = work.tile([C, L], BF16, tag="rden", bufs=2)
            _act_recip(nc, rden_t[:], den_t[:])
            wn_t = work.tile([C, L], BF16, tag="wn", bufs=6)
            eng_of("wn_eng").tensor_tensor(wn_t[:], wsq_t[:], rden_t[:], AOP.mult)
            wn_l[n] = wn_t
            # tmp chunk A (L-cols 0:1024) accumulates into this group's ptmpA
            pta = ptmpA[g]
            for (o, cw) in [(0, 512), (512, 512)]:
                nc.tensor.matmul(
                    pta[0:32, o : o + cw],
                    m64_s[:, n * 32 : (n + 1) * 32],
                    wn_t[:, o : o + cw],
                    start=(j == 0),
                    stop=(j == GS - 1),
                )

        def exp_a(g):
            """exp over tmp chunk A straight from psum (no max subtraction)."""
            r0, r1 = g * 32, (g + 1) * 32
            e_g = soft.tile([32, L], BF16, tag="e", bufs=2)
            sA = soft.tile([32, 1], F32, tag="sA", bufs=2)
            e_l[g], sA_l[g] = e_g, sA
            nc.scalar.activation(
                e_g[:, 0:1024],
                ptmpA[g][0:32, 0:1024],
                AFT.Exp,
                scale=sc_s[r0:r1, :],
                accum_out=sA[:],
            )

        def tmp_b_wave(g):
            """tmp chunk B (L-cols 1024:1600) matmuls + exp."""
            r0, r1 = g * 32, (g + 1) * 32
            ptb = pspool.tile([32, 1024], F32, tag="ptmp", bufs=2)
            for j in range(GS):
                n = g * GS + j
                for (o, cw, po) in [(1024, 512, 0), (1536, 64, 512)]:
                    nc.tensor.matmul(
                        ptb[0:32, po : po + cw],
                        m64_s[:, n * 32 : (n + 1) * 32],
                        wn_l[n][:, o : o + cw],
                        start=(j == 0),
                        stop=(j == GS - 1),
                    )
            sB = soft.tile([32, 1], F32, tag="sB", bufs=2)
            sB_l[g] = sB
            nc.scalar.activation(
                e_l[g][:, 1024:L],
                ptb[0:32, 0:576],
                AFT.Exp,
                scale=sc_s[r0:r1, :],
                accum_out=sB[:],
            )

        def b_small(g):
            """Softmax epilogue in the unnormalized-e domain.

            With the 1e-8 regularizers dropped (they perturb y only where
            cumPi <~ 1e-6, i.e. |y| <~ |w|*1e-6 -- far below tolerance):
              y = w*Pi*cumPi/D = w*e*t_b/D0hat,
              t = e*cumE*rs,  D0hat = cumsum((1+wsq)*e)  (c-side scan).
            So the c-side needs only e (early) and t (late) -- broadcast
            separately so the c-chain starts as soon as e is ready."""
            e_g = e_l[g]
            s_g = soft.tile([32, 1], F32, tag="s", bufs=2)
            nc.vector.tensor_tensor(s_g[:], sA_l[g][:], sB_l[g][:], AOP.add)
            rs = soft.tile([32, 1], F32, tag="rs", bufs=2)
            nc.vector.reciprocal(rs[:], s_g[:])
            t_g = soft.tile([32, L], BF16, tag="t", bufs=2)
            nc.vector.tensor_tensor_scan(t_g[:], e_g[:], z32L, 0.0, AOP.add, AOP.add)
            nc.vector.tensor_scalar_mul(t_g[:], t_g[:], rs[:])
            nc.vector.tensor_tensor(t_g[:], t_g[:], e_g[:], AOP.mult)
            return t_g

        eb_l = [None] * NLOC
        tb_l = [None] * NLOC
        y_l = [None] * NLOC

        def c_bcast_e(n):
            """Early head-broadcast of e for sample n (feeds p2m + D-scan).
            Dedicated sync DGE ring so e never queues behind t/out traffic."""
            r = 8 * (n % GS)
            e_g = e_l[n // GS]
            eb = work.tile([C, L], BF16, tag="eb", bufs=6)
            eng = (nc.sync, nc.scalar, nc.gpsimd)[n % 3]
            eng.dma_start(
                eb[:], e_g[r : r + 8, :].unsqueeze(1).broadcast_to((8, HD, L))
            )
            eb_l[n] = eb

        def c_bcast_t(n, t_g):
            """Late head-broadcast of t = e*cumE/s (feeds v2 only)."""
            r = 8 * (n % GS)
            tb = work.tile([C, L], BF16, tag="tb", bufs=6)
            eng = (nc.gpsimd, nc.sync, nc.scalar)[n % 3]
            eng.dma_start(
                tb[:], t_g[r : r + 8, :].unsqueeze(1).broadcast_to((8, HD, L))
            )
            tb_l[n] = tb

        rD_l = [None] * NLOC

        def c_pd(n):
            """p2m + D-scan + reciprocal for sample n (needs only eb)."""
            Eb = eb_l[n][:]
            p2m = work.tile([C, L], BF16, tag="p2m", bufs=2)
            eng_of("p2m_eng").tensor_tensor(p2m[:], wsq_l[n][:], Eb, AOP.mult)
            D_t = work.tile([C, L], BF16, tag="D", bufs=2)
            nc.vector.tensor_tensor_scan(
                D_t[:], p2m[:], Eb, 0.0, AOP.add, AOP.add
            )
            rD = work.tile([C, L], BF16, tag="rD", bufs=3)
            _act_recip(nc, rD[:], D_t[:])
            rD_l[n] = rD

        def c_vy(n):
            """v2 + y for sample n (needs tb, which arrives later)."""
            v2 = work.tile([C, L], BF16, tag="v2", bufs=2)
            eng_of("v2_eng").tensor_tensor(v2[:], w_l[n][:], tb_l[n][:], AOP.mult)
            y_t = work.tile([C, L], BF16, tag="y", bufs=3)
            eng_of("y_eng").tensor_tensor(y_t[:], v2[:], rD_l[n][:], AOP.mult)
            y_l[n] = y_t

        def c_out(n):
            """Projection + residual + relu + store for sample n."""
            y_t = y_l[n]
            out_sb = opool.tile([C, L], FP16, tag="outsb", bufs=2)
            o_n = n * L
            for (ho, hw, subs) in HALVES:
                pj = pspool.tile([C, 1024], F32, tag="pp", bufs=2)
                for (so, sw) in subs:
                    nc.tensor.matmul(
                        pj[:, so : so + sw],
                        wptn_s[:],
                        y_t[:, ho + so : ho + so + sw],
                        start=True,
                        stop=False,
                    )
                for (so, sw) in subs:
                    nc.tensor.matmul(
                        pj[:, so : so + sw],
                        iden_s[:],
                        x_all[:, o_n + ho + so : o_n + ho + so + sw],
                        start=False,
                        stop=True,
                    )
                nc.scalar.activation(
                    out_sb[:, ho : ho + hw], pj[:, 0:hw], AFT.Relu, bias=bp_s[:]
                )
            eng = nc.sync if n % 2 == 0 else nc.gpsimd
            eng.dma_start(out_d[n], out_sb[:])

        # Software-pipelined emission in data-readiness order: all A-phases
        # lead; each group's softmax/broadcast chain threads between later
        # A-pieces so broadcast latency always overlaps independent work,
        # and c-chains trail where their inputs become ready.
        for n in range(5):
            a_one(n)
        exp_a(0)
        a_one(5)
        tmp_b_wave(0)
        a_one(6)
        for n in range(4):
            c_bcast_e(n)
        t0 = b_small(0)
        for n in range(4):
            c_bcast_t(n, t0)
        a_one(7)
        exp_a(1)
        tmp_b_wave(1)
        for n in range(4, 8):
            c_bcast_e(n)
        t1 = b_small(1)
        for n in range(4, 8):
            c_bcast_t(n, t1)
        c_pd(0)
        c_pd(1)
        c_vy(0)
        c_pd(2)
        c_vy(1)
        c_out(0)
        c_pd(3)
        c_vy(2)
        c_out(1)
        c_pd(4)
        c_vy(3)
        c_out(2)
        c_pd(5)
        c_vy(4)
        c_out(3)
        c_pd(6)
        c_vy(5)
        c_out(4)
        c_pd(7)
        c_vy(6)
        c_out(5)
        c_vy(7)
        c_out(6)
        c_out(7)

    nc.compile()
    return nc


def make_core_inputs(inputs, cfg=None):
    """Host-side prep: returns (shared_map, per_core_x_list)."""
    x = np.asarray(inputs["x"], np.float32)  # (N,C,T,V)
    Wa = np.asarray(inputs["Wa"], np.float32)
    ba = np.asarray(inputs["ba"], np.float32)
    Wp = np.asarray(inputs["Wp"], np.float32)
    bp = np.asarray(inputs["bp"], np.float32)
    temp = np.asarray(inputs["temp"], np.float32).reshape(H)
    # denom_bias adds a per-(n,h) constant to the softmax logits -> cancels.

    assert np.all(temp > 0), "kernel assumes temp > 0"
    assert temp.max() * 16.0 < 80.0, "kernel assumes exp(16*temp) fits fp32"

    xr = np.ascontiguousarray(x.reshape(N, C, L).astype(np.float16))
    wat16 = np.ascontiguousarray(Wa.T).astype(np.float16)
    wptn_bf = np.ascontiguousarray((-Wp.T)).astype(ml_dtypes.bfloat16)
    iden16 = np.eye(C, dtype=np.float16)
    m64 = np.zeros((C, NLOC * 32), np.float32)
    cc = np.arange(C)
    for n in range(NLOC):
        m64[cc, n * 32 + 8 * (n % GS) + cc // HD] = 1.0
    m64bf = m64.astype(ml_dtypes.bfloat16)
    pp = np.arange(64)
    sc64 = temp[pp % 8].reshape(64, 1).astype(np.float32)

    shared = dict(
        wat16=wat16,
        wptn_bf=wptn_bf,
        iden16=iden16,
        ba=ba.reshape(C, 1),
        bp=bp.reshape(C, 1),
        m64bf=m64bf,
        sc64=sc64,
    )
    xs = [np.ascontiguousarray(xr[i * NLOC : (i + 1) * NLOC]) for i in range(NCORES)]
    return shared, xs


_NC_CACHE = {}


def kernel(**inputs):
    cfg_key = "default"
    if cfg_key not in _NC_CACHE:
        _NC_CACHE[cfg_key] = build_nc()
    nc = _NC_CACHE[cfg_key]
    shared, xs = make_core_inputs(inputs)
    in_maps = [dict(shared, x16=xs[i]) for i in range(NCORES)]
    res = run_bass_kernel_spmd(nc, in_maps, core_ids=list(range(NCORES)))
    out = np.concatenate([res.results[i]["out16"] for i in range(NCORES)], axis=0)
    return out.reshape(N, C, T, V).astype(np.float32)


if __name__ == "__main__":
    rng = np.random.default_rng(0)
    demo = dict(
        x=rng.standard_normal((N, C, T, V)).astype(np.float32),
        Wa=rng.standard_normal((C, C)).astype(np.float32) / np.sqrt(C),
        ba=rng.standard_normal((C,)).astype(np.float32) * 0.01,
        Wp=rng.standard_normal((C, C)).astype(np.float32) / np.sqrt(C),
        bp=rng.standard_normal((C,)).astype(np.float32) * 0.01,
        temp=np.ones((H, 1), np.float32),
        denom_bias=np.zeros((H, 1, 1), np.float32),
    )
    o = kernel(**demo)
    print("out", o.shape, o.dtype, float(np.abs(o).max()))
